# revision 27
# baseline (speedup 1.0000x reference)
"""Trainium2 Bass kernel for nn_Network_76493367542190 (HRR network).

Math (derived from the reference, validated in numpy):
  - binding/unbinding along E are circulant matmuls: x @ A with
    A[n,m] = ef[(m-n)%E] (bind) / df[(n-m)%E] (unbind).
  - the FFT seq-conv reduces to a 32-tap depthwise circular conv along S
    scaled by sqrt(S); the `+ x1*w` gate folds into tap 0.
  - per-layer LN folds: the centering term is a constant offset of the
    circulant unbind matrix (Au' = Au - sum(df)/E), so no rank-1
    correction matmuls are needed; ln_scale folds into dW rows; ln_bias
    folds into the dense bias.

Distribution: data-parallel over batch, 2 batches per core on 8 cores.
Device layout: activations transposed [E_chunk(6) x 128, token], bf16,
fp32 PSUM accumulation.

The depthwise conv is split across PE (per-tap diagonal matmuls
accumulating shifted slices in PSUM; chunks 0-2) and DVE
(scalar_tensor_tensor FMA chains into fp32 SBUF accumulators; chunks
3-5) so the tensor engine no longer serializes the whole conv; GPSIMD
has no scalar_tensor_tensor opcode, so it instead takes the two-tensor
elementwise work (x4 scale, GLU product, skip add, halo copies). The
whole layer is emitted as a per-tile software pipeline
(conv(t) | unbind(t-1) | LN-scale+dense(t-2)) because each engine
drains its queue in program order.

Host does: embedding gather (with mask folded into row 0 of the table),
LN0, transpose to device layout, and the tiny final pooled->logits
matmul + log_softmax. Device does the 4 layers + masked-sum pooling.
"""
import numpy as np
import ml_dtypes

B, S, V, E, L, O = 16, 2048, 32000, 768, 4, 10
KLEN = 32
EPS = 1e-6
NCORES = 8
BPC = B // NCORES          # batches per core
NB = BPC                   # 2
TPB = S                    # tokens per batch
T = NB * TPB               # tokens per core
HALO = 32
BSTRIDE = TPB + HALO       # 2080
TT = 512                   # token tile
QPB = TPB // TT            # 4 tiles per batch
NT = NB * QPB              # 8 token tiles per core
EC = E // 128              # 6 e-chunks
FC = 2 * E // 128          # 12 dense out chunks
BFNP = ml_dtypes.bfloat16

# conv engine assignment: engine for each (chunk, tile) unit. GPSIMD has
# no scalar_tensor_tensor opcode on TRN2, so the conv FMA chains run on
# PE (diag matmuls, ~8.3us/unit) and DVE (~18.7us/unit) only, balanced
# against PE's fixed matmul load; GPSIMD instead takes over the
# two-tensor elementwise ops (x4 scale, GLU product, skip add).
def _conv_engine(c, t):
    if c <= 2:
        return "PE"
    return "DVE"

CONV_PE_CHUNKS = (0, 1, 2)

# bind emission order: each batch's last tile first, so the circular
# halo (copied from the batch tail) is ready before conv starts.
BIND_ORDER = (3, 0, 1, 2, 7, 4, 5, 6)

_STATE = {}


# ---------------------------------------------------------------- device build

def _build(n_layers=L, repeat=1):
    import concourse.mybir as mybir
    import concourse.tile as tile
    from concourse import bacc
    from contextlib import ExitStack

    dt = mybir.dt
    f32 = dt.float32
    bf16 = dt.bfloat16
    AF = mybir.ActivationFunctionType
    OP = mybir.AluOpType

    nc = bacc.Bacc("TRN2", target_bir_lowering=False, debug=False)

    xin = nc.dram_tensor("xin", [EC, 128, T], bf16, kind="ExternalInput").ap()
    bindW = nc.dram_tensor("bindW", [L, EC, 128, E], bf16, kind="ExternalInput").ap()
    unbW = nc.dram_tensor("unbW", [L, EC, 128, E], bf16, kind="ExternalInput").ap()
    denseW = nc.dram_tensor("denseW", [L, EC, 128, 2 * E], bf16, kind="ExternalInput").ap()
    denseB = nc.dram_tensor("denseB", [L, FC, 128, 1], f32, kind="ExternalInput").ap()
    tapsI = nc.dram_tensor("taps", [L, EC, 128, KLEN], f32, kind="ExternalInput").ap()
    identI = nc.dram_tensor("ident", [128, 128], bf16, kind="ExternalInput").ap()
    ones128I = nc.dram_tensor("ones128", [1, 128], bf16, kind="ExternalInput").ap()
    onescolI = nc.dram_tensor("onescol", [128, 1], bf16, kind="ExternalInput").ap()
    maskI = nc.dram_tensor("maskb", [NB, 128, TPB], bf16, kind="ExternalInput").ap()
    pooled = nc.dram_tensor("pooled", [EC, 128, NB], f32, kind="ExternalOutput").ap()
    # Internal-kind DRAM tensors hang the PJRT/axon execute path, so the
    # skip staging lives in (unread) ExternalOutput tensors instead.
    skipd = [
        nc.dram_tensor(f"skipd{i}", [EC, 128, T], bf16, kind="ExternalOutput").ap()
        for i in range(2)
    ]

    def dslice(buf, t, w=TT):
        b, q = divmod(t, QPB)
        s = b * BSTRIDE + HALO + q * TT
        return buf[:, s:s + w]

    def bslice(buf, b):
        s = b * BSTRIDE + HALO
        return buf[:, s:s + TPB]

    def cslice(buf, t, tap):
        b, q = divmod(t, QPB)
        s = b * BSTRIDE + HALO + q * TT - tap
        return buf[:, s:s + TT]

    with tile.TileContext(nc) as tc, ExitStack() as ctx:
        persist = ctx.enter_context(tc.tile_pool(name="persist", bufs=1))
        # persistent activation buffers (A: x_in/x2/x_out, B: x1ext/x4)
        Abuf, Bbuf = [], []
        for c in range(EC):
            a = persist.tile([128, NB * BSTRIDE], bf16, tag=f"A{c}", name=f"actA{c}")
            b_ = persist.tile([128, NB * BSTRIDE], bf16, tag=f"B{c}", name=f"actB{c}")
            Abuf.append(a)
            Bbuf.append(b_)

        ident_t = persist.tile([128, 128], bf16, tag="ident", name="ident_t")
        ones128_t = persist.tile([1, 128], bf16, tag="ones128", name="ones128_t")
        onescol_t = persist.tile([128, 1], bf16, tag="onescol", name="onescol_t")
        eps_t = persist.tile([1, 1], f32, tag="eps", name="eps_t")
        nc.sync.dma_start(out=ident_t, in_=identI)
        nc.sync.dma_start(out=ones128_t, in_=ones128I)
        nc.sync.dma_start(out=onescol_t, in_=onescolI)
        nc.vector.memset(eps_t, EPS)

        mask_t = []
        for b in range(NB):
            m = persist.tile([128, TPB], bf16, tag=f"mask{b}", name=f"mask{b}")
            nc.sync.dma_start(out=m, in_=maskI[b])
            mask_t.append(m)

        wpool = ctx.enter_context(tc.tile_pool(name="weights", bufs=1))
        dpool = ctx.enter_context(tc.tile_pool(name="diags", bufs=1))
        stg = ctx.enter_context(tc.tile_pool(name="staging", bufs=1))
        rows = ctx.enter_context(tc.tile_pool(name="rows", bufs=2))
        accp = ctx.enter_context(tc.tile_pool(name="accs", bufs=1))
        psmm = ctx.enter_context(tc.tile_pool(name="psmm", bufs=6, space="PSUM"))
        psrow = ctx.enter_context(tc.tile_pool(name="psrow", bufs=2, space="PSUM"))

        # load xin into A
        for c in range(EC):
            for b in range(NB):
                nc.sync.dma_start(out=bslice(Abuf[c], b),
                                  in_=xin[c, :, b * TPB:(b + 1) * TPB])

        for pos in range(n_layers * repeat):
            l = pos % n_layers
            # ---- per-layer weights
            bw = []
            uw = []
            dw = []
            tp = []
            for c in range(EC):
                w1 = wpool.tile([128, E], bf16, tag=f"bw{c}", name=f"bw{l}_{c}")
                nc.sync.dma_start(out=w1, in_=bindW[l, c])
                bw.append(w1)
                w2 = wpool.tile([128, E], bf16, tag=f"uw{c}", name=f"uw{l}_{c}")
                nc.sync.dma_start(out=w2, in_=unbW[l, c])
                uw.append(w2)
                w3 = wpool.tile([128, 2 * E], bf16, tag=f"dw{c}", name=f"dw{l}_{c}")
                nc.sync.dma_start(out=w3, in_=denseW[l, c])
                dw.append(w3)
                w4 = wpool.tile([128, KLEN], f32, tag=f"tp{c}", name=f"tp{l}_{c}")
                nc.sync.dma_start(out=w4, in_=tapsI[l, c])
                tp.append(w4)
            dbt = []
            for fc in range(FC):
                bcol = wpool.tile([128, 1], f32, tag=f"dbb{fc}", bufs=2,
                                  name=f"dbb{l}_{fc}")
                nc.sync.dma_start(out=bcol, in_=denseB[l, fc])
                dbt.append(bcol)

            # diagonal tap weights for the PE conv chunks
            dg = {}
            for c in CONV_PE_CHUNKS:
                for tap in range(KLEN):
                    d = dpool.tile([128, 128], bf16, tag=f"dg{c}_{tap}",
                                   name=f"dg{l}_{c}_{tap}")
                    nc.vector.tensor_scalar_mul(d, ident_t, tp[c][:, tap:tap + 1])
                    dg[(c, tap)] = d

            def bind(t):
                # x1 = (x @ A)^T -> B (with halo layout)
                for eo in range(EC):
                    ps = psmm.tile([128, TT], f32, tag="mm", name=f"bps{l}_{t}_{eo}")
                    for k in range(EC):
                        nc.tensor.matmul(ps, lhsT=bw[k][:, eo * 128:(eo + 1) * 128],
                                         rhs=dslice(Abuf[k], t),
                                         start=(k == 0), stop=(k == EC - 1))
                    nc.scalar.copy(dslice(Bbuf[eo], t), ps)
                # circular halo as soon as a batch's tail tile lands
                b, q = divmod(t, QPB)
                if q == QPB - 1:
                    for c in range(EC):
                        nc.gpsimd.tensor_copy(
                            Bbuf[c][:, b * BSTRIDE:b * BSTRIDE + HALO],
                            Bbuf[c][:, b * BSTRIDE + TPB:b * BSTRIDE + TPB + HALO])

            def conv(t):
                # 32-tap depthwise circular conv + gelu -> A (=x2):
                # PE runs diag matmuls for its chunks, DVE runs FMA chains
                # into fp32 accumulators for the rest.
                for c in range(EC):
                    if _conv_engine(c, t) == "PE":
                        ps = psmm.tile([128, TT], f32, tag="mm", name=f"cps{l}_{c}_{t}")
                        for tap in range(KLEN):
                            nc.tensor.matmul(ps, lhsT=dg[(c, tap)],
                                             rhs=cslice(Bbuf[c], t, tap),
                                             start=(tap == 0), stop=(tap == KLEN - 1))
                        nc.scalar.activation(dslice(Abuf[c], t), ps, AF.Gelu_apprx_tanh)
                    else:
                        acc = accp.tile([128, TT], f32, tag="dacc", bufs=2,
                                        name=f"dacc{l}_{c}_{t}")
                        nc.vector.tensor_scalar_mul(acc, cslice(Bbuf[c], t, 0),
                                                    tp[c][:, 0:1])
                        for tap in range(1, KLEN):
                            nc.vector.scalar_tensor_tensor(
                                acc, cslice(Bbuf[c], t, tap), tp[c][:, tap:tap + 1],
                                acc, OP.mult, OP.add)
                        nc.scalar.activation(dslice(Abuf[c], t), acc,
                                             AF.Gelu_apprx_tanh)

            ss_live = {}

            def unbind_mm(t):
                # unbind (centering folded into uw): matmul groups + psum
                # downcast (s) and square (sq) staging on ACT
                sqs = []
                ss = []
                for eo in range(EC):
                    ps = psmm.tile([128, TT], f32, tag="mm", name=f"ups{l}_{t}_{eo}")
                    for k in range(EC):
                        nc.tensor.matmul(ps, lhsT=uw[k][:, eo * 128:(eo + 1) * 128],
                                         rhs=dslice(Abuf[k], t),
                                         start=(k == 0), stop=(k == EC - 1))
                    s = stg.tile([128, TT], bf16, tag=f"s{eo}", bufs=2,
                                 name=f"s{l}_{t}_{eo}")
                    nc.scalar.copy(s, ps)
                    sq = stg.tile([128, TT], bf16, tag="sq", bufs=7,
                                  name=f"sq{l}_{t}_{eo}")
                    nc.scalar.activation(sq, ps, AF.Square)
                    ss.append(s)
                    sqs.append(sq)
                ss_live[t] = (ss, sqs)

            def unbind_var(t):
                # variance accumulation + 1/sigma (emitted at stage end so
                # PE reaches it only after ACT produced the sq tiles)
                ss, sqs = ss_live[t]
                psv = psrow.tile([1, TT], f32, tag="row", name=f"vsp{l}_{t}")
                for eo in range(EC):
                    nc.tensor.matmul(psv, lhsT=onescol_t, rhs=sqs[eo],
                                     start=(eo == 0), stop=(eo == EC - 1))
                albf = rows.tile([1, TT], bf16, tag="albf", name=f"albf{l}_{t}")
                nc.scalar.activation(albf, psv, AF.Abs_reciprocal_sqrt,
                                     bias=eps_t, scale=1.0 / E)
                ss_live[t] = (ss, albf)

            def unbind_fin(t):
                # broadcast 1/sigma and apply the LN scale -> B (=x4)
                ss, albf = ss_live.pop(t)
                psb = psmm.tile([128, TT], f32, tag="mm", name=f"abp{l}_{t}")
                nc.tensor.matmul(psb, lhsT=ones128_t, rhs=albf, start=True, stop=True)
                ab = stg.tile([128, TT], bf16, tag="ab", bufs=2, name=f"ab{l}_{t}")
                nc.scalar.copy(ab, psb)
                for eo in range(EC):
                    nc.gpsimd.tensor_mul(dslice(Bbuf[eo], t), ss[eo], ab)

            def dense(t):
                # dense + GLU + skip -> A (=x_out)
                for fp in range(EC):
                    psa = psmm.tile([128, TT], f32, tag="mm", name=f"da{l}_{t}_{fp}")
                    for k in range(EC):
                        nc.tensor.matmul(psa, lhsT=dw[k][:, fp * 128:(fp + 1) * 128],
                                         rhs=dslice(Bbuf[k], t),
                                         start=(k == 0), stop=(k == EC - 1))
                    psg = psmm.tile([128, TT], f32, tag="mm", name=f"db{l}_{t}_{fp}")
                    for k in range(EC):
                        nc.tensor.matmul(psg,
                                         lhsT=dw[k][:, (fp + EC) * 128:(fp + EC + 1) * 128],
                                         rhs=dslice(Bbuf[k], t),
                                         start=(k == 0), stop=(k == EC - 1))
                    sig = stg.tile([128, TT], bf16, tag="sig", bufs=2,
                                   name=f"sig{l}_{t}_{fp}")
                    nc.scalar.activation(sig, psg, AF.Sigmoid, bias=dbt[fp + EC])
                    sa = stg.tile([128, TT], bf16, tag="sa", bufs=2,
                                  name=f"sa{l}_{t}_{fp}")
                    nc.scalar.activation(sa, psa, AF.Identity, bias=dbt[fp])
                    prod = stg.tile([128, TT], bf16, tag="pr", bufs=2,
                                    name=f"pr{l}_{t}_{fp}")
                    nc.gpsimd.tensor_mul(prod, sa, sig)
                    skt = stg.tile([128, TT], bf16, tag="skt", bufs=3,
                                   name=f"skt{pos}_{t}_{fp}")
                    if pos == 0:
                        nc.sync.dma_start(out=skt, in_=xin[fp, :, t * TT:(t + 1) * TT])
                    else:
                        nc.sync.dma_start(out=skt,
                                          in_=skipd[(pos - 1) % 2][fp, :, t * TT:(t + 1) * TT])
                    nc.gpsimd.tensor_add(dslice(Abuf[fp], t), prod, skt)
                    if pos < n_layers * repeat - 1:
                        nc.sync.dma_start(out=skipd[pos % 2][fp, :, t * TT:(t + 1) * TT],
                                          in_=dslice(Abuf[fp], t))

            # software-pipelined emission: engine queues are serviced in
            # program order, so interleave the stages per token tile to
            # keep all engines' queues stocked with ready work.
            for t in BIND_ORDER:
                bind(t)
            for t in range(NT + 2):
                if t < NT:
                    conv(t)
                if 1 <= t <= NT:
                    unbind_mm(t - 1)
                if t >= 2:
                    unbind_fin(t - 2)
                    dense(t - 2)
                if 1 <= t <= NT:
                    unbind_var(t - 1)

        # ---- masked-sum pooling
        for c in range(EC):
            for b in range(NB):
                pr = stg.tile([128, TPB], bf16, tag="poolscratch", bufs=1,
                              name=f"ppr{c}_{b}")
                acc = rows.tile([128, 1], f32, tag=f"acc", bufs=4, name=f"acc{c}_{b}")
                nc.vector.tensor_mul(pr, bslice(Abuf[c], b), mask_t[b])
                nc.vector.reduce_sum(acc, pr, axis=mybir.AxisListType.X)
                nc.sync.dma_start(out=pooled[c, :, b:b + 1], in_=acc)

    nc.compile()
    return nc


def _get_nc(n_layers=L, repeat=1):
    key = ("nc", n_layers, repeat)
    if key not in _STATE:
        _STATE[key] = _build(n_layers, repeat)
    return _STATE[key]


# ---------------------------------------------------------------- host side

def _host_prep(inputs):
    f32 = np.float32
    enc = np.asarray(inputs["encoder_input"])
    embed = np.asarray(inputs["embed"], f32)
    ln0_scale = np.asarray(inputs["ln0_scale"], f32)
    ln0_bias = np.asarray(inputs["ln0_bias"], f32)
    ef = np.asarray(inputs["ef"], f32)
    cf = np.asarray(inputs["cf"], f32)
    df = np.asarray(inputs["df"], f32)
    w = np.asarray(inputs["w"], f32)
    ln_scale = np.asarray(inputs["ln_scale"], f32)
    ln_bias = np.asarray(inputs["ln_bias"], f32)
    dW = np.asarray(inputs["dW"], f32)
    db = np.asarray(inputs["db"], f32)

    # --- shared weights
    n = np.arange(E)
    bidx = (n[None, :] - n[:, None]) % E          # A[n,m] = ef[(m-n)%E]
    uidx = (n[:, None] - n[None, :]) % E          # Au[n,m] = df[(n-m)%E]
    bindW = np.empty((L, EC, 128, E), dtype=BFNP)
    unbW = np.empty((L, EC, 128, E), dtype=BFNP)
    denseW = np.empty((L, EC, 128, 2 * E), dtype=BFNP)
    denseB = np.empty((L, FC, 128, 1), dtype=np.float32)
    taps = np.empty((L, EC, 128, KLEN), dtype=np.float32)
    sqS = f32(np.sqrt(np.float64(S)))
    for l in range(L):
        A = ef[l][bidx]
        # LN centering folds into the unbind circulant as a constant
        # offset: x4C[e] = sum_m x2[m] (df[(m-e)%E] - sum(df)/E).
        gamma = f32(-np.sum(df[l], dtype=np.float64) / E)
        Au = df[l][uidx] + gamma
        dWf = dW[l] * ln_scale[l][:, None]
        bpp = dW[l].T @ ln_bias[l] + db[l]
        c2 = (sqS * cf[l]).astype(f32)
        c2[0, :] = c2[0, :] + w[l]
        for c in range(EC):
            r = slice(c * 128, (c + 1) * 128)
            bindW[l, c] = A[r].astype(BFNP)
            unbW[l, c] = Au[r].astype(BFNP)
            denseW[l, c] = dWf[r].astype(BFNP)
            taps[l, c] = c2[:, r].T.astype(BFNP).astype(np.float32)
        denseB[l] = bpp.astype(BFNP).astype(np.float32).reshape(FC, 128, 1)
    ident = np.eye(128, dtype=BFNP)
    ones128 = np.ones((1, 128), dtype=BFNP)
    onescol = np.ones((128, 1), dtype=BFNP)

    # --- embedding + LN0 on host
    emb2 = embed.copy()
    emb2[0, :] = 0.0
    mask_full = (enc > 0).astype(f32)             # [B,S]

    in_maps = []
    for core in range(NCORES):
        encl = enc[core * BPC:(core + 1) * BPC]            # [2, S]
        x0 = emb2[encl]                                    # [2, S, E] f32
        mu = x0.mean(-1, keepdims=True)
        var = x0.var(-1, keepdims=True)
        x0 = (x0 - mu) / np.sqrt(var + EPS) * ln0_scale + ln0_bias
        xin = np.ascontiguousarray(
            x0.reshape(T, E).T).reshape(EC, 128, T).astype(BFNP)
        maskl = mask_full[core * BPC:(core + 1) * BPC]     # [2, S]
        maskb = np.ascontiguousarray(
            np.broadcast_to(maskl[:, None, :], (NB, 128, TPB))).astype(BFNP)
        in_maps.append({
            "xin": xin, "bindW": bindW, "unbW": unbW, "denseW": denseW,
            "denseB": denseB, "taps": taps, "ident": ident,
            "ones128": ones128, "onescol": onescol,
            "maskb": maskb,
        })
    return in_maps, mask_full


def _host_epilogue(results, mask_full, inputs):
    f32 = np.float32
    outW = np.asarray(inputs["outW"], f32)
    outb = np.asarray(inputs["outb"], f32)
    pooled = np.empty((B, E), f32)
    for core in range(NCORES):
        p = results[core]["pooled"]                        # [EC,128,NB] f32
        for b in range(NB):
            pooled[core * BPC + b] = p[:, :, b].reshape(E)
    nmask = mask_full.sum(1)                               # [B]
    pooled = pooled / nmask[:, None]
    out = pooled @ outW + outb
    m = out.max(-1, keepdims=True)
    lse = np.log(np.exp(out - m).sum(-1, keepdims=True)) + m
    return (out - lse).astype(f32)


def run_device(inputs, trace=False, n_layers=L, repeat=1):
    from concourse import bass_utils
    in_maps, mask_full = _host_prep(inputs)
    nc = _get_nc(n_layers, repeat)
    res = bass_utils.run_bass_kernel_spmd(
        nc, in_maps, core_ids=list(range(NCORES)), trace=trace)
    out = _host_epilogue(res.results, mask_full, inputs)
    return out, res


def _agree(pa, pb):
    return all(
        np.allclose(a, b, rtol=2e-3, atol=1e-3) for a, b in zip(pa, pb)
    )


def kernel(**inputs) -> np.ndarray:
    # Rarely (~1 in 10 executes) a run through the tunneled PJRT path
    # returns corrupted results. Correct runs reproduce bit-stably, so
    # run the device pass twice and accept two agreeing runs (third run
    # as tiebreaker).
    from concourse import bass_utils
    in_maps, mask_full = _host_prep(inputs)
    nc = _get_nc()
    attempts = []
    for _ in range(3):
        res = bass_utils.run_bass_kernel_spmd(
            nc, in_maps, core_ids=list(range(NCORES)))
        attempts.append([np.array(r["pooled"]) for r in res.results])
        for i in range(len(attempts) - 1):
            if _agree(attempts[i], attempts[-1]):
                results = [{"pooled": p} for p in attempts[-1]]
                return _host_epilogue(results, mask_full, inputs)
    results = [{"pooled": p} for p in attempts[-1]]
    return _host_epilogue(results, mask_full, inputs)
